# revision 2
# baseline (speedup 1.0000x reference)
"""MultiReDiffusion Trainium2 kernel.

Math (per batch b, relation r):
    coef[r]   = sum_s theta[r,s] * t[r,s]
    diff      = A[b,r] @ X[b,r]                       # [N, Din]
    fc        = coef[r] * diff @ W[r] + fc_b[r]       # [N, Dout]
    h         = prelu(fc, p0)
    mixed[q]  = sum_r conv_w[q,r] * h[r] + conv_b[q]
    out[q]    = prelu(mixed[q], p1)

Device strategy (one batch element per NeuronCore, 8 cores):
  * Fold coef and W into the moving operand: Y[r] = coef[r] * (X[r] @ W[r]),
    so the heavy matmul is A @ Y (associativity; A@X@W == A@(X@W)).
  * Host pre-transposes A to A^T ([r, m, n]) so the contraction index m
    lands on SBUF partitions; PE computes psum[n,e] += At_blk.T @ Y_blk
    with At blocks as the stationary operand (no on-device transposes).
  * fc_b enters PSUM via a K=1 matmul with a ones row vector.
  * Epilogue per 128-row tile: prelu0 (ACT relu + DVE fused min*p + add),
    4x4 relation mix as 4 broadcast multiplies + tree adds, conv_b add,
    prelu1, then DMA out.
The kernel is HBM-bound: each core streams 64 MiB of A^T once.
"""

import os

import numpy as np

import concourse.bass as bass
import concourse.mybir as mybir
import concourse.tile as tile
from concourse.tile import TileContext
from concourse.bass_utils import run_bass_kernel_spmd

F32 = mybir.dt.float32
P = 128
B, R, N, DIN, DOUT = 8, 4, 2048, 32, 32
MB = N // P          # contraction blocks
NT = N // P          # output row tiles
GRP = 4              # row tiles processed per PSUM generation
NGRP = NT // GRP
QE = 4 * DOUT        # mixed free width (q, e)

last_exec_time_ns = None


def _split_multi_waits(nc, max_waits=1):
    """This walrus build rejects instructions carrying more than one
    sync-wait ("Too many sync wait commands" in setupSyncWait).  Hoist
    excess waits onto same-engine no-ops inserted just before the
    offending instruction — semantically identical (all waits are
    monotonic sem-ge preconditions)."""
    ctr = 0
    for f in nc.m.functions:
        for bb in f.blocks:
            rebuilt = []
            changed = False
            for ins in bb.instructions:
                si = ins.sync_info
                waits = list(si.on_wait) if si is not None else []
                if len(waits) > max_waits:
                    changed = True
                    for w in waits[:-max_waits]:
                        ctr += 1
                        rebuilt.append(
                            mybir.InstNoOp(
                                name=f"waitsplit-{ctr}",
                                engine=ins.engine,
                                sync_info=mybir.SyncInfo(on_wait=[w], on_update=[]),
                            )
                        )
                    ins.sync_info = mybir.SyncInfo(
                        on_wait=waits[-max_waits:], on_update=list(si.on_update)
                    )
                rebuilt.append(ins)
            if changed:
                bb.instructions = rebuilt


def _patched_drain_and_barrier(self, tick_clock, wait_clock):
    """Tail drain with its waits split onto single-wait SP nops (the
    multi-wait Drain is what trips walrus first)."""
    from bass_rust import ScopedClock

    nc = self.nc
    collect = nc.sync.nop(nofuse=True)
    wait_clock.add_sem_waits(collect.ins, ScopedClock({None: tick_clock.global_clock}))
    si = collect.ins.sync_info
    waits = list(si.on_wait) if si is not None else []
    if len(waits) > 1:
        collect.ins.sync_info = mybir.SyncInfo(on_wait=waits[:1], on_update=[])
        for w in waits[1:]:
            n = nc.sync.nop(nofuse=True)
            n.ins.sync_info = mybir.SyncInfo(on_wait=[w], on_update=[])
    nc.sync.drain()
    nc.all_engine_barrier()
    assert self.sems is not None
    popped = nc._tile_sem_poison_stack.pop()
    assert popped is self._sem_poison
    nc.clear_and_free_semaphores(list(self.sems.allocated().values()))
    nc.all_engine_barrier()


tile.TileContext._drain_and_barrier = _patched_drain_and_barrier

_PROGRAM = None


def _build_program():
    global _PROGRAM
    if _PROGRAM is not None:
        return _PROGRAM

    nc = bass.Bass("TRN2", target_bir_lowering=False, debug=False, num_devices=8)
    at = nc.dram_tensor("at", [R, N, N], F32, kind="ExternalInput")
    yt = nc.dram_tensor("yt", [P, R * MB * DOUT], F32, kind="ExternalInput")
    fcb = nc.dram_tensor("fcb", [1, R * DOUT], F32, kind="ExternalInput")
    wbv = nc.dram_tensor("wb", [P, R * QE], F32, kind="ExternalInput")
    cbv = nc.dram_tensor("cb", [P, QE], F32, kind="ExternalInput")
    p01 = nc.dram_tensor("p01", [P, 2], F32, kind="ExternalInput")
    out = nc.dram_tensor("out", [N, QE], F32, kind="ExternalOutput")

    Alu = mybir.AluOpType
    Act = mybir.ActivationFunctionType

    with TileContext(nc) as tc:
        with (
            tc.tile_pool(name="consts", bufs=1) as cpool,
            tc.tile_pool(name="strips", bufs=12) as spool,
            tc.tile_pool(name="psum", bufs=8, space="PSUM") as ppool,
            tc.tile_pool(name="work", bufs=3) as wpool,
        ):
            ytile = cpool.tile([P, R * MB * DOUT], F32, tag="yt")
            nc.sync.dma_start(out=ytile[:], in_=yt[:])
            fcbt = cpool.tile([1, R * DOUT], F32, tag="fcb")
            nc.sync.dma_start(out=fcbt[:], in_=fcb[:])
            wbt = cpool.tile([P, R * QE], F32, tag="wb")
            nc.sync.dma_start(out=wbt[:], in_=wbv[:])
            cbt = cpool.tile([P, QE], F32, tag="cb")
            nc.sync.dma_start(out=cbt[:], in_=cbv[:])
            pt = cpool.tile([P, 2], F32, tag="p01")
            nc.sync.dma_start(out=pt[:], in_=p01[:])
            ones = cpool.tile([1, P], F32, tag="ones")
            nc.vector.memset(ones[:], 1.0)

            for g in range(NGRP):
                n0 = g * GRP * P
                pss = []
                for t in range(GRP):
                    ps = ppool.tile([P, R * DOUT], F32, tag="ps")
                    pss.append(ps)
                    for r in range(R):
                        # start=True zeroes the whole 2 KiB PSUM bank (the
                        # zero-region is bank-sized on trn2), so only the
                        # first matmul into this tile may carry it.
                        nc.tensor.matmul(
                            ps[:, r * DOUT : (r + 1) * DOUT],
                            ones[:, :],
                            fcbt[:, r * DOUT : (r + 1) * DOUT],
                            start=(r == 0),
                            stop=False,
                        )
                for r in range(R):
                    for mb in range(MB):
                        strip = spool.tile([P, GRP * P], F32, tag="strip")
                        nc.sync.dma_start(
                            out=strip[:],
                            in_=at[r, mb * P : (mb + 1) * P, n0 : n0 + GRP * P],
                        )
                        yv = ytile[
                            :, r * MB * DOUT + mb * DOUT : r * MB * DOUT + (mb + 1) * DOUT
                        ]
                        for t in range(GRP):
                            nc.tensor.matmul(
                                pss[t][:, r * DOUT : (r + 1) * DOUT],
                                strip[:, t * P : (t + 1) * P],
                                yv,
                                start=False,
                                stop=(mb == MB - 1),
                            )
                for t in range(GRP):
                    ps = pss[t]
                    t1 = wpool.tile([P, R * DOUT], F32, tag="t1")
                    nc.scalar.activation(out=t1[:], in_=ps[:], func=Act.Relu)
                    t2 = wpool.tile([P, R * DOUT], F32, tag="t2")
                    nc.vector.tensor_scalar(
                        t2[:], ps[:], 0.0, pt[:, 0:1], Alu.min, Alu.mult
                    )
                    h = wpool.tile([P, R * DOUT], F32, tag="h")
                    nc.vector.tensor_add(h[:], t1[:], t2[:])

                    tms = []
                    for r in range(R):
                        tm = wpool.tile([P, QE], F32, tag=f"tm{r}")
                        hin = h[:, r * DOUT : (r + 1) * DOUT]
                        nc.vector.tensor_tensor(
                            out=tm[:].rearrange("p (q e) -> p q e", q=4),
                            in0=hin[:, None, :].to_broadcast([P, 4, DOUT]),
                            in1=wbt[:, r * QE : (r + 1) * QE].rearrange(
                                "p (q e) -> p q e", q=4
                            ),
                            op=Alu.mult,
                        )
                        tms.append(tm)
                    s01 = wpool.tile([P, QE], F32, tag="s01")
                    nc.vector.tensor_add(s01[:], tms[0][:], tms[1][:])
                    s23 = wpool.tile([P, QE], F32, tag="s23")
                    nc.vector.tensor_add(s23[:], tms[2][:], tms[3][:])
                    mix = wpool.tile([P, QE], F32, tag="mix")
                    nc.vector.tensor_add(mix[:], s01[:], s23[:])
                    mixb = wpool.tile([P, QE], F32, tag="mixb")
                    nc.vector.tensor_add(mixb[:], mix[:], cbt[:])

                    o1 = wpool.tile([P, QE], F32, tag="o1")
                    nc.scalar.activation(out=o1[:], in_=mixb[:], func=Act.Relu)
                    o2 = wpool.tile([P, QE], F32, tag="o2")
                    nc.vector.tensor_scalar(
                        o2[:], mixb[:], 0.0, pt[:, 1:2], Alu.min, Alu.mult
                    )
                    o = wpool.tile([P, QE], F32, tag="o")
                    nc.vector.tensor_add(o[:], o1[:], o2[:])
                    nt = g * GRP + t
                    nc.sync.dma_start(out=out[nt * P : (nt + 1) * P, :], in_=o[:])

    _split_multi_waits(nc)
    _PROGRAM = nc
    return nc


def kernel(
    theta_param,
    t_param,
    a_input_batched,
    x_input_batched,
    fc_w,
    fc_b,
    conv_w,
    conv_b,
    prelu0,
    prelu1,
):
    global last_exec_time_ns
    nc = _build_program()

    theta = np.asarray(theta_param, np.float32)
    tp = np.asarray(t_param, np.float32)
    A = np.asarray(a_input_batched, np.float32)
    X = np.asarray(x_input_batched, np.float32)
    W = np.asarray(fc_w, np.float32)
    fb = np.asarray(fc_b, np.float32)
    cw = np.asarray(conv_w, np.float32)
    cbp = np.asarray(conv_b, np.float32)
    p0 = np.asarray(prelu0, np.float32)
    p1 = np.asarray(prelu1, np.float32)

    coef = np.sum(theta * tp, axis=1)                      # [R]
    Y = np.matmul(X, W[None]) * coef[None, :, None, None]  # [B, R, N, DOUT]
    yt = (
        Y.reshape(B, R, MB, P, DOUT)
        .transpose(0, 3, 1, 2, 4)
        .reshape(B, P, R * MB * DOUT)
    )
    at = A.transpose(0, 1, 3, 2)                           # [B, R, m, n]

    fcb = np.ascontiguousarray(fb.reshape(1, R * DOUT))
    wb_flat = np.broadcast_to(cw.T[:, :, None], (R, 4, DOUT)).reshape(R * QE)
    wb = np.ascontiguousarray(np.broadcast_to(wb_flat[None, :], (P, R * QE)))
    cb_flat = np.broadcast_to(cbp[:, None], (4, DOUT)).reshape(QE)
    cb = np.ascontiguousarray(np.broadcast_to(cb_flat[None, :], (P, QE)))
    p01 = np.ascontiguousarray(
        np.broadcast_to(np.array([p0[0], p1[0]], np.float32)[None, :], (P, 2))
    )

    in_maps = [
        {
            "at": np.ascontiguousarray(at[b]),
            "yt": np.ascontiguousarray(yt[b]),
            "fcb": fcb,
            "wb": wb,
            "cb": cb,
            "p01": p01,
        }
        for b in range(B)
    ]

    trace = os.environ.get("BASS_KERNEL_TRACE") == "1"
    try:
        res = run_bass_kernel_spmd(nc, in_maps, list(range(B)), trace=trace)
    except Exception:
        if not trace:
            raise
        res = run_bass_kernel_spmd(nc, in_maps, list(range(B)), trace=False)
    last_exec_time_ns = res.exec_time_ns

    outs = []
    for b in range(B):
        o = res.results[b]["out"]  # [N, QE]
        outs.append(o.reshape(NT, P, 4, DOUT).transpose(2, 0, 1, 3).reshape(R, N, DOUT))
    return np.stack(outs, 0).astype(np.float32)


# revision 4
# speedup vs baseline: 1.9749x; 1.9749x over previous
"""MultiReDiffusion Trainium2 kernel.

Math (per batch b, relation r):
    coef[r]   = sum_s theta[r,s] * t[r,s]
    diff      = A[b,r] @ X[b,r]                       # [N, Din]
    fc        = coef[r] * diff @ W[r] + fc_b[r]       # [N, Dout]
    h         = prelu(fc, p0)
    mixed[q]  = sum_r conv_w[q,r] * h[r] + conv_b[q]
    out[q]    = prelu(mixed[q], p1)

Device strategy (one batch element per NeuronCore, 8 cores):
  * Fold coef and W into the moving operand: Y[r] = coef[r] * (X[r] @ W[r]),
    so the heavy matmul is A @ Y (associativity; A@X@W == A@(X@W)).
  * Host pre-transposes A to A^T ([r, m, n]) so the contraction index m
    lands on SBUF partitions.  The kernel computes the TRANSPOSED output
    psum[e, n] += Y[r,mb].T @ At[r, mb-block, :] with the tiny Y blocks
    as the stationary operand (32-column weight loads) and wide 512-col
    A^T strips as the moving operand — the PE streams ~512 moving
    columns per cheap weight load instead of paying a 128x128 fp32
    weight load per 32 moving columns.
  * PE column-tiling (tile_position=(0, 32r)) packs all four relations
    into one PSUM bank: relation r lands on PSUM partitions 32r..32r+31.
    With partitions = (r, e), fc_b is a per-partition constant (free via
    the ACT bias operand) and the 4x4 relation mix becomes ONE matmul
    with a kron(conv_w.T, I32) stationary.
  * A^T is streamed as 64 x 1 MiB row-contiguous DMAs (8 KiB per
    partition) — the kernel is HBM-bound at ~64 MiB per core.
"""

import os

import numpy as np

import concourse.bass as bass
import concourse.mybir as mybir
import concourse.tile as tile
from concourse.tile import TileContext
from concourse.bass_utils import run_bass_kernel_spmd

F32 = mybir.dt.float32
P = 128
B, R, N, DIN, DOUT = 8, 4, 2048, 32, 32
MB = N // P          # contraction blocks per relation
NC = 4               # n-chunks (PSUM bank = 512 fp32 per partition)
CW = N // NC         # chunk width (512)
QE = 4 * DOUT        # packed (q, e) partition width = 128

last_exec_time_ns = None


def _split_multi_waits(nc, max_waits=1):
    """This walrus build rejects instructions carrying more than one
    sync-wait ("Too many sync wait commands" in setupSyncWait).  Hoist
    excess waits onto same-engine no-ops inserted just before the
    offending instruction — semantically identical (all waits are
    monotonic sem-ge preconditions)."""
    ctr = 0
    for f in nc.m.functions:
        for bb in f.blocks:
            rebuilt = []
            changed = False
            for ins in bb.instructions:
                si = ins.sync_info
                waits = list(si.on_wait) if si is not None else []
                if len(waits) > max_waits:
                    changed = True
                    for w in waits[:-max_waits]:
                        ctr += 1
                        rebuilt.append(
                            mybir.InstNoOp(
                                name=f"waitsplit-{ctr}",
                                engine=ins.engine,
                                sync_info=mybir.SyncInfo(on_wait=[w], on_update=[]),
                            )
                        )
                    ins.sync_info = mybir.SyncInfo(
                        on_wait=waits[-max_waits:], on_update=list(si.on_update)
                    )
                rebuilt.append(ins)
            if changed:
                bb.instructions = rebuilt


def _patched_drain_and_barrier(self, tick_clock, wait_clock):
    """Tail drain with its waits split onto single-wait SP nops (the
    multi-wait Drain is what trips walrus first)."""
    from bass_rust import ScopedClock

    nc = self.nc
    collect = nc.sync.nop(nofuse=True)
    wait_clock.add_sem_waits(collect.ins, ScopedClock({None: tick_clock.global_clock}))
    si = collect.ins.sync_info
    waits = list(si.on_wait) if si is not None else []
    if len(waits) > 1:
        collect.ins.sync_info = mybir.SyncInfo(on_wait=waits[:1], on_update=[])
        for w in waits[1:]:
            n = nc.sync.nop(nofuse=True)
            n.ins.sync_info = mybir.SyncInfo(on_wait=[w], on_update=[])
    nc.sync.drain()
    nc.all_engine_barrier()
    assert self.sems is not None
    popped = nc._tile_sem_poison_stack.pop()
    assert popped is self._sem_poison
    nc.clear_and_free_semaphores(list(self.sems.allocated().values()))
    nc.all_engine_barrier()


tile.TileContext._drain_and_barrier = _patched_drain_and_barrier

_PROGRAM = None


def _build_program():
    global _PROGRAM
    if _PROGRAM is not None:
        return _PROGRAM

    nc = bass.Bass("TRN2", target_bir_lowering=False, debug=False, num_devices=8)
    at = nc.dram_tensor("at", [R, N, N], F32, kind="ExternalInput")
    yt = nc.dram_tensor("yt", [P, R * MB * DOUT], F32, kind="ExternalInput")
    mkv = nc.dram_tensor("mker", [P, P], F32, kind="ExternalInput")
    # per-partition constant columns: 0=fc_b[(r e)], 1=p0, 2=p1, 3=conv_b[(q e)]
    ccv = nc.dram_tensor("ccols", [P, 4], F32, kind="ExternalInput")
    out = nc.dram_tensor("out", [R, DOUT, N], F32, kind="ExternalOutput")

    Alu = mybir.AluOpType
    Act = mybir.ActivationFunctionType

    with TileContext(nc) as tc:
        with (
            tc.tile_pool(name="consts", bufs=1) as cpool,
            tc.tile_pool(name="strips", bufs=8) as spool,
            tc.tile_pool(name="psacc", bufs=4, space="PSUM") as ppool,
            tc.tile_pool(name="psmix", bufs=2, space="PSUM") as mpool,
            tc.tile_pool(name="work", bufs=3) as wpool,
        ):
            ytile = cpool.tile([P, R * MB * DOUT], F32, tag="yt")
            nc.sync.dma_start(out=ytile[:], in_=yt[:])
            mker = cpool.tile([P, P], F32, tag="mker")
            nc.sync.dma_start(out=mker[:], in_=mkv[:])
            cc = cpool.tile([P, 4], F32, tag="ccols")
            nc.sync.dma_start(out=cc[:], in_=ccv[:])
            fcb_c, p0_c, p1_c, cb_c = (cc[:, i : i + 1] for i in range(4))

            # Accumulators: pt[c][32r:32r+32, :] holds (A@Y)[r]^T for
            # n-columns [c*512, (c+1)*512).
            pts = [
                ppool.tile([P, CW], F32, tag="pt", name=f"pt{c}") for c in range(NC)
            ]

            for r in range(R):
                for mb in range(MB):
                    strip = spool.tile([P, N], F32, tag="strip")
                    nc.sync.dma_start(
                        out=strip[:], in_=at[r, mb * P : (mb + 1) * P, :]
                    )
                    yv = ytile[
                        :, r * MB * DOUT + mb * DOUT : r * MB * DOUT + (mb + 1) * DOUT
                    ]
                    for c in range(NC):
                        # start=True zeroes the whole 2 KiB PSUM bank
                        # (zero-region = bank on trn2): only the first
                        # matmul into each bank carries it.
                        nc.tensor.matmul(
                            pts[c][32 * r : 32 * (r + 1), :],
                            yv,
                            strip[:, c * CW : (c + 1) * CW],
                            start=(r == 0 and mb == 0),
                            stop=(mb == MB - 1),
                            tile_position=(0, 32 * r),
                        )

            for c in range(NC):
                pt = pts[c]
                # h = prelu(pt + fc_b, p0), partitions = (r, e)
                t1 = wpool.tile([P, CW], F32, tag="t1")
                nc.scalar.activation(out=t1[:], in_=pt[:], func=Act.Relu, bias=fcb_c)
                t2 = wpool.tile([P, CW], F32, tag="t2")
                nc.vector.tensor_scalar(t2[:], pt[:], fcb_c, 0.0, Alu.add, Alu.min)
                h = wpool.tile([P, CW], F32, tag="h")
                nc.vector.scalar_tensor_tensor(
                    h[:], t2[:], p0_c, t1[:], Alu.mult, Alu.add
                )
                # mixed^T[(q e'), n] = kron(conv_w.T, I32).T @ h
                p2 = mpool.tile([P, CW], F32, tag="p2")
                nc.tensor.matmul(p2[:], mker[:], h[:], start=True, stop=True)
                # out = prelu(mixed + conv_b, p1), partitions = (q, e)
                o1 = wpool.tile([P, CW], F32, tag="o1")
                nc.scalar.activation(out=o1[:], in_=p2[:], func=Act.Relu, bias=cb_c)
                o2 = wpool.tile([P, CW], F32, tag="o2")
                nc.vector.tensor_scalar(o2[:], p2[:], cb_c, 0.0, Alu.add, Alu.min)
                o = wpool.tile([P, CW], F32, tag="o")
                nc.vector.scalar_tensor_tensor(
                    o[:], o2[:], p1_c, o1[:], Alu.mult, Alu.add
                )
                nc.sync.dma_start(
                    out=out[:, :, c * CW : (c + 1) * CW], in_=o[:]
                )

    _split_multi_waits(nc)
    _PROGRAM = nc
    return nc


def kernel(
    theta_param,
    t_param,
    a_input_batched,
    x_input_batched,
    fc_w,
    fc_b,
    conv_w,
    conv_b,
    prelu0,
    prelu1,
):
    global last_exec_time_ns
    nc = _build_program()

    theta = np.asarray(theta_param, np.float32)
    tp = np.asarray(t_param, np.float32)
    A = np.asarray(a_input_batched, np.float32)
    X = np.asarray(x_input_batched, np.float32)
    W = np.asarray(fc_w, np.float32)
    fb = np.asarray(fc_b, np.float32)
    cw = np.asarray(conv_w, np.float32)
    cbp = np.asarray(conv_b, np.float32)
    p0 = np.asarray(prelu0, np.float32)
    p1 = np.asarray(prelu1, np.float32)

    coef = np.sum(theta * tp, axis=1)                      # [R]
    Y = np.matmul(X, W[None]) * coef[None, :, None, None]  # [B, R, N, DOUT]
    yt = (
        Y.reshape(B, R, MB, P, DOUT)
        .transpose(0, 3, 1, 2, 4)
        .reshape(B, P, R * MB * DOUT)
    )
    at = A.transpose(0, 1, 3, 2)                           # [B, R, m, n]

    mker = np.ascontiguousarray(np.kron(cw.T, np.eye(DOUT, dtype=np.float32)))
    ccols = np.stack(
        [
            fb.reshape(QE),                                        # fc_b[(r e)]
            np.full(QE, p0[0], np.float32),
            np.full(QE, p1[0], np.float32),
            np.broadcast_to(cbp[:, None], (4, DOUT)).reshape(QE),  # conv_b[(q e)]
        ],
        axis=1,
    ).astype(np.float32)

    in_maps = [
        {
            "at": np.ascontiguousarray(at[b]),
            "yt": np.ascontiguousarray(yt[b]),
            "mker": mker,
            "ccols": ccols,
        }
        for b in range(B)
    ]

    trace = os.environ.get("BASS_KERNEL_TRACE") == "1"
    try:
        res = run_bass_kernel_spmd(nc, in_maps, list(range(B)), trace=trace)
    except Exception:
        if not trace:
            raise
        res = run_bass_kernel_spmd(nc, in_maps, list(range(B)), trace=False)
    last_exec_time_ns = res.exec_time_ns

    outs = []
    for b in range(B):
        o = res.results[b]["out"]  # [R(q), DOUT(e), N(n)]
        outs.append(o.transpose(0, 2, 1))
    return np.ascontiguousarray(np.stack(outs, 0)).astype(np.float32)
